# revision 1
# baseline (speedup 1.0000x reference)
"""GQA causal attention with RoPE (B=8, S=1024, D=2048, 16 Q / 4 KV heads)
on 8 trn2 NeuronCores, data-parallel over batch (1 batch element per core).

All matmuls run in bf16 (fp32 PSUM accumulation). Layout notes:
- Head dim is de-interleaved on the host (real parts in [0:64), imag in
  [64:128) of each head block) by permuting wq/wk rows, which turns RoPE
  into contiguous half-block ops on the device.
- Scores are computed transposed (ST[u, t]) so that softmax(P)@V needs no
  on-device transposition of P; causal masking is a PE-additive -1e30
  strict-lower mask on the diagonal 128x128 blocks; softmax denominators
  come from an all-ones stationary matmul (every PSUM row = column sum).
"""
import os
import sys

sys.path.insert(0, "/opt/trn_rl_repo")

import numpy as np
import ml_dtypes

BF16NP = ml_dtypes.bfloat16

DIM = 2048
NH = 16
NKV = 4
HD = 128
B = 8
S = 1024
NT = S // 128          # 8 query/key tiles
ND = DIM // 128        # 16 contraction chunks
SCALE = 1.0 / float(np.sqrt(HD))

_CACHE = {}
LAST_EXEC_NS = None


def _build_nc():
    import concourse.mybir as mybir
    import concourse.tile as tile
    from concourse import bacc
    from contextlib import ExitStack

    F32 = mybir.dt.float32
    BF = mybir.dt.bfloat16
    EXP = mybir.ActivationFunctionType.Exp
    MULT = mybir.AluOpType.mult
    ADD = mybir.AluOpType.add

    nc = bacc.Bacc()
    xb = nc.declare_dram_parameter("xb", [S, DIM], BF, isOutput=False)
    wqt = nc.declare_dram_parameter("wqt", [DIM, DIM], BF, isOutput=False)
    wkt = nc.declare_dram_parameter("wkt", [DIM, 512], BF, isOutput=False)
    wvt = nc.declare_dram_parameter("wvt", [DIM, 512], BF, isOutput=False)
    wot = nc.declare_dram_parameter("wot", [DIM, DIM], BF, isOutput=False)
    ropec = nc.declare_dram_parameter("ropec", [S, 128], BF, isOutput=False)
    ropes = nc.declare_dram_parameter("ropes", [S, 128], BF, isOutput=False)
    ident = nc.declare_dram_parameter("ident", [128, 128], BF, isOutput=False)
    onesm = nc.declare_dram_parameter("onesm", [128, 128], BF, isOutput=False)
    lmask = nc.declare_dram_parameter("lmask", [128, 128], BF, isOutput=False)
    outp = nc.declare_dram_parameter("out", [S, DIM], F32, isOutput=True)

    def r4(ap):
        return ap.rearrange("p (h x b) -> p h x b", h=4, x=2, b=64)

    with tile.TileContext(nc) as tc, ExitStack() as ctx:
        pool = lambda name, bufs, **kw: ctx.enter_context(
            tc.tile_pool(name=name, bufs=bufs, **kw))

        const = pool("const", 1)
        p_xt = pool("p_xt", 1)
        p_kn = pool("p_kn", 1)
        p_v = pool("p_v", 1)
        p_kt = pool("p_kt", 1)
        p_qn = pool("p_qn", 2)
        p_qt = pool("p_qt", 3)
        p_pt = pool("p_pt", 4)
        p_ot = pool("p_ot", 1)
        p_tmp = pool("p_tmp", 2)
        p_rd = pool("p_rd", 2)
        p_ob = pool("p_ob", 2)

        ps_mm = pool("ps_mm", 4, space="PSUM")
        ps_ot = pool("ps_ot", 2, space="PSUM")
        ps_dn = pool("ps_dn", 2, space="PSUM")

        # ---- constants (identity first: transposes need it immediately) ----
        ident_s = const.tile([128, 128], BF, name="ident_s")
        nc.gpsimd.dma_start(out=ident_s[:], in_=ident[:])

        XT = p_xt.tile([128, ND, S], BF, name="XT")  # XT[p, dc, t] = x[t, dc*128+p]

        # transient weight pools, LIFO-nested: p_wq opens first (closes last)
        ctx_wq = tc.tile_pool(name="p_wq", bufs=2)
        p_wq = ctx_wq.__enter__()
        wq_tiles = [None] * NKV

        def load_wq(g):
            t = p_wq.tile([128, ND, 512], BF, name="wqt_g")
            nc.gpsimd.dma_start(
                out=t[:],
                in_=wqt[:, g * 512:(g + 1) * 512].rearrange(
                    "(dc p) o -> p dc o", p=128))
            wq_tiles[g] = t

        ctx_wkv = tc.tile_pool(name="p_wkv", bufs=1)
        p_wkv = ctx_wkv.__enter__()
        wkt_s = p_wkv.tile([128, ND, 512], BF, name="wkt_s")
        wvt_s = p_wkv.tile([128, ND, 512], BF, name="wvt_s")
        nc.gpsimd.dma_start(out=wkt_s[:],
                            in_=wkt[:].rearrange("(dc p) o -> p dc o", p=128))
        nc.gpsimd.dma_start(out=wvt_s[:],
                            in_=wvt[:].rearrange("(dc p) o -> p dc o", p=128))
        load_wq(0)

        # remaining constants
        ones_s = const.tile([128, 128], BF, name="ones_s")
        lmask_s = const.tile([128, 128], BF, name="lmask_s")
        ropec_s = const.tile([128, NT, 128], BF, name="ropec_s")
        ropes_s = const.tile([128, NT, 128], BF, name="ropes_s")
        nc.sync.dma_start(out=ones_s[:], in_=onesm[:])
        nc.sync.dma_start(out=lmask_s[:], in_=lmask[:])
        nc.sync.dma_start(out=ropec_s[:],
                          in_=ropec[:].rearrange("(j p) o -> p j o", p=128))
        nc.sync.dma_start(out=ropes_s[:],
                          in_=ropes[:].rearrange("(j p) o -> p j o", p=128))

        Knat = p_kn.tile([128, NT, 512], BF, name="Knat")
        V = p_v.tile([128, NT, 512], BF, name="V")

        def rope_j(nat, j):
            # q' = q * C2 + swap_halves(q) * S2, operating on 4 head blocks
            t1 = p_tmp.tile([128, 512], BF, name="ropet1")
            t2 = p_tmp.tile([128, 512], BF, name="ropet2")
            a4 = r4(nat[:, j, :])
            s_b = ropes_s[:, j, :].rearrange("p (x b) -> p x b", x=2) \
                .unsqueeze(1).broadcast_to((128, 4, 2, 64))
            c_b = ropec_s[:, j, :].unsqueeze(1).broadcast_to((128, 4, 128))
            nc.vector.tensor_tensor(out=r4(t1[:]), in0=a4[:, :, ::-1, :],
                                    in1=s_b, op=MULT)
            nc.vector.tensor_tensor(
                out=t2[:].rearrange("p (h o) -> p h o", h=4),
                in0=nat[:, j, :].rearrange("p (h o) -> p h o", h=4),
                in1=c_b, op=MULT)
            nc.vector.tensor_tensor(out=nat[:, j, :], in0=t1[:], in1=t2[:], op=ADD)

        with tc.tile_pool(name="p_xload", bufs=2) as p_xload:
            for tj in range(NT):
                xt_in = p_xload.tile([128, DIM], BF, name="xt_in")
                if tj == 0:  # quarter-loads so the first transposes start early
                    for q in range(4):
                        nc.sync.dma_start(
                            out=xt_in[:, q * 512:(q + 1) * 512],
                            in_=xb[tj * 128:(tj + 1) * 128, q * 512:(q + 1) * 512])
                else:
                    nc.sync.dma_start(out=xt_in[:],
                                      in_=xb[tj * 128:(tj + 1) * 128, :])
                for dc4 in range(4):
                    ps = ps_mm.tile([128, 512], BF, name="ps_mm_t")
                    for k in range(4):
                        dc = dc4 * 4 + k
                        nc.tensor.transpose(ps[:, k * 128:(k + 1) * 128],
                                            xt_in[:, dc * 128:(dc + 1) * 128],
                                            ident_s[:])
                    nc.any.tensor_copy(
                        out=XT[:, dc4 * 4:dc4 * 4 + 4, tj * 128:tj * 128 + 128],
                        in_=ps[:].rearrange("p (k t) -> p k t", k=4))
                psk = ps_mm.tile([128, 512], F32, name="ps_mm_t")
                for dc in range(ND):
                    nc.tensor.matmul(psk[:], XT[:, dc, tj * 128:(tj + 1) * 128],
                                     wkt_s[:, dc, :],
                                     start=(dc == 0), stop=(dc == ND - 1))
                nc.any.tensor_copy(out=Knat[:, tj, :], in_=psk[:])
                rope_j(Knat, tj)
        for j in range(NT):
            psv = ps_mm.tile([128, 512], F32, name="ps_mm_t")
            for dc in range(ND):
                nc.tensor.matmul(psv[:], XT[:, dc, j * 128:(j + 1) * 128],
                                 wvt_s[:, dc, :],
                                 start=(dc == 0), stop=(dc == ND - 1))
            nc.any.tensor_copy(out=V[:, j, :], in_=psv[:])
        ctx_wkv.__exit__(None, None, None)

        KT = []
        for g in range(NKV):
            ktg = p_kt.tile([128, S], BF, name=f"KT{g}")
            KT.append(ktg)
            for j4 in range(2):
                ps = ps_mm.tile([128, 512], BF, name="ps_mm_t")
                for k in range(4):
                    j = j4 * 4 + k
                    nc.tensor.transpose(ps[:, k * 128:(k + 1) * 128],
                                        Knat[:, j, g * 128:(g + 1) * 128],
                                        ident_s[:])
                nc.any.tensor_copy(out=ktg[:, j4 * 512:(j4 + 1) * 512], in_=ps[:])

        # ---- phase C: per group, Q proj + rope + per-head attention ----
        # Group g+1's Q projection/rope is pipelined inside group g's head
        # loop (2 t-tiles per head) so the proj->copy->rope->transpose chain
        # never surfaces at group boundaries.
        OT = []  # OT[h]: [128(dv), S] bf16
        qnats = [None] * NKV

        def qproj_tj(g, tj):
            psq = ps_mm.tile([128, 512], F32, name="ps_mm_t")
            for dc in range(ND):
                nc.tensor.matmul(psq[:], XT[:, dc, tj * 128:(tj + 1) * 128],
                                 wq_tiles[g][:, dc, :],
                                 start=(dc == 0), stop=(dc == ND - 1))
            nc.any.tensor_copy(out=qnats[g][:, tj, :], in_=psq[:])
            rope_j(qnats[g], tj)

        qnats[0] = p_qn.tile([128, NT, 512], BF, name="Qnat")
        load_wq(1)
        for tj in range(NT):
            qproj_tj(0, tj)

        for g in range(NKV):
            Qnat = qnats[g]
            if g + 1 < NKV:
                qnats[g + 1] = p_qn.tile([128, NT, 512], BF, name="Qnat")
                if g + 2 < NKV:
                    load_wq(g + 2)

            for hh in range(4):
                h = 4 * g + hh
                qt_h = p_qt.tile([128, S], BF, name="qt_h")
                for j4 in range(2):
                    ps = ps_mm.tile([128, 512], BF, name="ps_mm_t")
                    for k in range(4):
                        j = j4 * 4 + k
                        nc.tensor.transpose(ps[:, k * 128:(k + 1) * 128],
                                            Qnat[:, j, hh * 128:(hh + 1) * 128],
                                            ident_s[:])
                    nc.any.tensor_copy(out=qt_h[:, j4 * 512:(j4 + 1) * 512],
                                       in_=ps[:])

                # attention for head h (kv group g), scores transposed ST[u, t]
                pot = [ps_ot.tile([128, 512], F32, name="ps_oth")
                       for _ in range(2)]
                pdn = [ps_dn.tile([128, 512], F32, name="ps_dnh")
                       for _ in range(2)]
                for j in range(NT):
                    if j < 4:
                        chunks = [(128 * j, 512 - 128 * j), (512, 512)]
                    else:
                        chunks = [(128 * j, 1024 - 128 * j)]
                    Pt = p_pt.tile([128, S], BF, name="ptile")
                    for ci, (t0, w) in enumerate(chunks):
                        pss = ps_mm.tile([128, 512], F32, name="ps_mm_t")
                        nc.tensor.matmul(pss[:, :w],
                                         KT[g][:, j * 128:(j + 1) * 128],
                                         qt_h[:, t0:t0 + w],
                                         start=True, stop=(ci != 0))
                        if ci == 0:  # diagonal block gets -1e30 strict-lower
                            nc.tensor.matmul(pss[:, :128], ident_s[:],
                                             lmask_s[:], start=False, stop=True,
                                             skip_group_check=True)
                        off = t0 - 128 * j
                        nc.scalar.activation(out=Pt[:, off:off + w],
                                             in_=pss[:, :w], func=EXP,
                                             scale=SCALE)
                    # PV + denominator accumulation per 512-half
                    for half in range(2):
                        if half == 0:
                            if j > 3:
                                continue
                            rsl = slice(0, 512 - 128 * j)
                            osl = slice(128 * j, 512)
                            start, stop = (j == 0), (j == 3)
                        else:
                            if j < 4:
                                rsl = slice(512 - 128 * j, 1024 - 128 * j)
                                osl = slice(0, 512)
                            else:
                                rsl = slice(0, 1024 - 128 * j)
                                osl = slice(128 * j - 512, 512)
                            start, stop = (j == 0), (j == NT - 1)
                        nc.tensor.matmul(pot[half][:, osl],
                                         V[:, j, g * 128:(g + 1) * 128],
                                         Pt[:, rsl], start=start, stop=stop,
                                         skip_group_check=True)
                        nc.tensor.matmul(pdn[half][:, osl], ones_s[:],
                                         Pt[:, rsl], start=start, stop=stop,
                                         skip_group_check=True)
                ot_h = p_ot.tile([128, S], BF, name=f"ot{h}")
                OT.append(ot_h)
                for half in range(2):
                    rd = p_rd.tile([128, 512], F32, name="rd")
                    nc.vector.reciprocal_approx_fast(out=rd[:], in_=pdn[half][:])
                    nc.vector.tensor_tensor(
                        out=ot_h[:, half * 512:(half + 1) * 512],
                        in0=pot[half][:], in1=rd[:], op=MULT)
                if g + 1 < NKV:
                    for tj in ([0, 1, 2], [3, 4, 5], [6, 7], [])[hh]:
                        qproj_tj(g + 1, tj)
        ctx_wq.__exit__(None, None, None)

        # ---- phase D: output projection ----
        with tc.tile_pool(name="p_wo", bufs=2) as p_wo:
            wo_tiles = [None] * 4

            def load_wo(oc):
                t = p_wo.tile([128, ND, 512], BF, name="wot_s")
                nc.gpsimd.dma_start(
                    out=t[:],
                    in_=wot[:, oc * 512:(oc + 1) * 512].rearrange(
                        "(dc p) o -> p dc o", p=128))
                wo_tiles[oc] = t

            load_wo(0)
            load_wo(1)
            for oc in range(4):
                wot_s = wo_tiles[oc]
                for tj in range(NT):
                    pso = ps_mm.tile([128, 512], F32, name="ps_mm_t")
                    for dc in range(ND):
                        nc.tensor.matmul(pso[:],
                                         OT[dc][:, tj * 128:(tj + 1) * 128],
                                         wot_s[:, dc, :],
                                         start=(dc == 0), stop=(dc == ND - 1))
                    ob = p_ob.tile([128, 512], F32, name="ob")
                    nc.any.tensor_copy(out=ob[:], in_=pso[:])
                    nc.sync.dma_start(
                        out=outp[tj * 128:(tj + 1) * 128,
                                 oc * 512:(oc + 1) * 512],
                        in_=ob[:])
                if oc + 2 < 4:
                    load_wo(oc + 2)

    nc.finalize()
    return nc


def _get_nc():
    if "nc" not in _CACHE:
        _CACHE["nc"] = _build_nc()
    return _CACHE["nc"]


def kernel(x, freqs_cos, freqs_sin, wq, wk, wv, wo):
    global LAST_EXEC_NS
    from concourse.bass_utils import run_bass_kernel_spmd

    x = np.asarray(x, dtype=np.float32)
    freqs_cos = np.asarray(freqs_cos, dtype=np.float32)
    freqs_sin = np.asarray(freqs_sin, dtype=np.float32)
    wq = np.asarray(wq, dtype=np.float32)
    wk = np.asarray(wk, dtype=np.float32)
    wv = np.asarray(wv, dtype=np.float32)
    wo = np.asarray(wo, dtype=np.float32)

    # de-interleave rope pairs: new head-dim index jj<64 -> old 2jj (real),
    # jj>=64 -> old 2(jj-64)+1 (imag)
    perm = np.concatenate([np.arange(0, HD, 2), np.arange(1, HD, 2)])
    wq_p = wq.reshape(NH, HD, DIM)[:, perm, :].reshape(NH * HD, DIM)
    wk_p = wk.reshape(NKV, HD, DIM)[:, perm, :].reshape(NKV * HD, DIM)

    def bfT(a):  # transpose to [DIM, out] and cast
        return np.ascontiguousarray(a.T).astype(BF16NP)

    shared = {
        "wqt": bfT(wq_p),
        "wkt": bfT(wk_p),
        "wvt": bfT(wv),
        "wot": np.ascontiguousarray(wo.T).astype(BF16NP),
        "ropec": np.concatenate([freqs_cos, freqs_cos], axis=1).astype(BF16NP),
        "ropes": np.concatenate([-freqs_sin, freqs_sin], axis=1).astype(BF16NP),
        "ident": np.eye(128, dtype=np.float32).astype(BF16NP),
        "onesm": np.ones((128, 128), np.float32).astype(BF16NP),
        "lmask": np.where(np.arange(128)[None, :] < np.arange(128)[:, None],
                          -1e30, 0.0).astype(np.float32).astype(BF16NP),
    }
    in_maps = [dict(shared, xb=x[b].astype(BF16NP)) for b in range(B)]

    nc = _get_nc()
    trace = bool(int(os.environ.get("KERNEL_TRACE", "0")))

    def _run(use_trace):
        return run_bass_kernel_spmd(nc, in_maps, core_ids=list(range(B)),
                                    trace=use_trace)

    try:
        res = _run(trace)
    except Exception:
        # trace hook missing, or transient device wedge: retry untraced once
        import time as _time
        _time.sleep(2.0)
        res = _run(False)
    LAST_EXEC_NS = getattr(res, "exec_time_ns", None)
    return np.stack([res.results[b]["out"] for b in range(B)], axis=0)

